# revision 15
# baseline (speedup 1.0000x reference)
"""L2 (spectral) contrastive loss on 8 Trainium2 NeuronCores.

Math: with G_x = x.T @ x and G_y = y.T @ y (both [D, D]),
    sum_{i,j} <x_i, y_j>^2 = ||x @ y.T||_F^2 = tr(G_x @ G_y) = sum(G_x * G_y)
so the loss needs only the two Gram matrices (2*N*D^2 MACs) instead of the
[N, N] pairwise product (N^2*D MACs) - a 5.3x FLOP reduction at N=8192, D=768.

Sharding: rows of x and y are split across the 8 cores. Each core computes
partial Grams over its 1024 rows (bf16 matmuls, fp32 PSUM accumulation,
upper-triangle slabs only - Grams are symmetric), plus per-partition partial
sums of the diagonal terms z_i = <x_i, y_i>. Partials are packed into one
fp16 buffer [128, 5392] and summed with a single AllReduce; every core then
computes
    loss = sum(G_x*G_y)/(N*(N-1)) - sum(z^2)/(N*(N-1)) - (2/N)*sum(z)
and core 0's output is returned.

Schedule notes (the AllReduce has ~27us of fixed firmware latency plus
~25us of transfer, so everything is arranged to ring its doorbell ASAP):
  - PE has DVFS p-states (full 2.4 GHz only after ~3us of gap-free
    execution), so the matmul queue must never stall once started. Both
    Grams use a diagonal-slot order: slab m consumes chunk k in round k+m,
    so early rounds track the DMA arrival rate, slabs close staggered
    (packs overlap the remaining rounds), and gram-y starts right as
    gram-x drains without waiting for a bulk pack phase.
  - Vector: fp32->bf16 casts of x (PE-critical path), gram-x packs,
    z STTs, the last gram-y pack.
  - Scalar/ACT: casts of y, gram-y packs m0..m4.
  - z partition-reduction is NOT done on-core: the scaled per-partition
    partials ride in two extra pack columns and the AllReduce sums them;
    post-collective they join the dot total in one ones^T matmul.
  - cin is staged to DRAM in 4 column chunks as packs complete, so the
    doorbell waits only on the last slab pack + a ~200KB DMA.
  - Post-collective: 4 chunked readbacks, dot(G_x, G_y) on vector (diag
    tiles once, strict-upper twice), one ones^T matmul reduces the dot
    total and both z scalars across partitions together.
"""
import numpy as np
from contextlib import ExitStack

from concourse import bacc, tile, mybir
from concourse.bass_utils import run_bass_kernel_spmd

N_CORES = 8
N, D = 8192, 768
ROWS = N // N_CORES          # 1024 rows per core
P = 128                      # SBUF partitions
KCH = ROWS // P              # 8 contraction chunks per core
MS = D // P                  # 6 output slabs per Gram

# upper-triangle slab widths and packed column offsets
WIDTHS = [D - P * m for m in range(MS)]              # [768,640,512,384,256,128]
COFF = [sum(WIDTHS[:m]) for m in range(MS)]          # prefix offsets
GCOLS = sum(WIDTHS)                                  # 2688 per Gram
PACK_COLS = 2 * GCOLS + 16                           # + scalar/pad region
SCAL_COL = 2 * GCOLS                                 # z partials at [:, SCAL:+2]
# bank-aligned PSUM slab allocations (bank = 512 fp32)
PSUM_PAD = [1024, 1024, 512, 512, 512, 512]

F32 = mybir.dt.float32
F16 = mybir.dt.float16
BF16 = mybir.dt.bfloat16

_CACHE = {}


def _free_chunks(width):
    """Split [0, width) at the 512-column PSUM bank boundary."""
    if width <= 512:
        return [(0, width)]
    return [(0, 512), (512, width)]


def _build():
    nc = bacc.Bacc("TRN2", target_bir_lowering=False, debug=False,
                   num_devices=N_CORES)
    x_ap = nc.dram_tensor("x", [ROWS, D], F32, kind="ExternalInput").ap()
    y_ap = nc.dram_tensor("y", [ROWS, D], F32, kind="ExternalInput").ap()
    loss_ap = nc.dram_tensor("loss", [1, 1], F32, kind="ExternalOutput").ap()

    inv_nn1 = 1.0 / (float(N) * (N - 1))
    add = mybir.AluOpType.add
    mult = mybir.AluOpType.mult
    AX = mybir.AxisListType.X

    with tile.TileContext(nc) as tc:
        with ExitStack() as ctx:
            sb = ctx.enter_context(tc.tile_pool(name="sb", bufs=1))
            ps = ctx.enter_context(tc.tile_pool(name="ps", bufs=1, space="PSUM"))
            dram = ctx.enter_context(tc.tile_pool(name="dram", bufs=1, space="DRAM"))

            # ---- load inputs: [1024, 768] -> [128p, 8k, 768], x before y ----
            xt = sb.tile([P, KCH, D], F32)
            yt = sb.tile([P, KCH, D], F32)
            xr = x_ap.rearrange("(n p) d -> p n d", p=P)
            yr = y_ap.rearrange("(n p) d -> p n d", p=P)
            for k in range(KCH):
                nc.sync.dma_start(xt[:, k, :], xr[:, k, :])
            for k in range(KCH):
                nc.sync.dma_start(yt[:, k, :], yr[:, k, :])

            # ---- small init on vector ----
            pack = sb.tile([P, PACK_COLS], F16)
            ones = sb.tile([P, 1], F32)
            nc.vector.memset(ones[:], 1.0)
            nc.vector.memset(pack[:, SCAL_COL + 2:PACK_COLS], 0.0)

            # ---- casts to bf16: x on vector (fast, PE-critical),
            # y on the scalar/ACT engine ----
            xb = sb.tile([P, KCH, D], BF16)
            yb = sb.tile([P, KCH, D], BF16)
            for k in range(KCH):
                nc.vector.tensor_copy(xb[:, k, :], xt[:, k, :])
            for k in range(KCH):
                nc.scalar.copy(yb[:, k, :], yt[:, k, :])

            # ---- PSUM slabs: one per m, bank-aligned, all 8 banks ----
            slabs = [
                ps.tile([P, WIDTHS[m]], F32,
                        padded_shape=[P, PSUM_PAD[m]], name=f"slab{m}")
                for m in range(MS)
            ]

            # ---- gram-x: diagonal-slot order (slab m eats chunk k in
            # round k+m) - arrival-paced, staggered closes, no PE stalls ----
            for s in range(KCH + MS - 1):
                for m in range(MS):
                    k = s - m
                    if 0 <= k < KCH:
                        for (c0, c1) in _free_chunks(WIDTHS[m]):
                            nc.tensor.matmul(
                                slabs[m][:, c0:c1],
                                xb[:, k, P * m:P * (m + 1)],
                                xb[:, k, P * m + c0:P * m + c1],
                                start=(k == 0),
                                stop=(k == KCH - 1),
                            )
            # pack gram-x on vector as slabs close
            for m in range(MS):
                nc.vector.tensor_copy(pack[:, COFF[m]:COFF[m] + WIDTHS[m]],
                                      slabs[m][:, 0:WIDTHS[m]])

            # ---- gram-y reusing the same slabs (WAR-ordered after packs) ----
            for s in range(KCH + MS - 1):
                for m in range(MS):
                    k = s - m
                    if 0 <= k < KCH:
                        for (c0, c1) in _free_chunks(WIDTHS[m]):
                            nc.tensor.matmul(
                                slabs[m][:, c0:c1],
                                yb[:, k, P * m:P * (m + 1)],
                                yb[:, k, P * m + c0:P * m + c1],
                                start=(k == 0),
                                stop=(k == KCH - 1),
                            )
            # pack gram-y: m0..m4 on ACT (free after casts), m5 on vector
            # (it gates the last cin chunk; vector copy is faster)
            for m in range(5):
                nc.scalar.copy(pack[:, GCOLS + COFF[m]:GCOLS + COFF[m] + WIDTHS[m]],
                               slabs[m][:, 0:WIDTHS[m]])

            # ---- z partials on vector: zred[p] = (sum_k z, sum_k z^2),
            # scaled and quantized per-partition; the AllReduce sums them ----
            zcols = sb.tile([P, KCH], F32)
            zscr = sb.tile([P, D], F32)
            for k in range(KCH):
                nc.vector.scalar_tensor_tensor(
                    zscr[:], xb[:, k, :], 1.0, yb[:, k, :],
                    mult, mult, accum_out=zcols[:, k:k + 1],
                )
            zsq = sb.tile([P, KCH], F32)
            nc.vector.tensor_mul(zsq[:], zcols[:], zcols[:])
            zred = sb.tile([P, 2], F32)
            nc.vector.tensor_reduce(zred[:, 0:1], zcols[:], AX, add)
            nc.vector.tensor_reduce(zred[:, 1:2], zsq[:], AX, add)
            nc.vector.tensor_scalar_mul(pack[:, SCAL_COL:SCAL_COL + 1],
                                        zred[:, 0:1], 2.0 / N)
            nc.vector.tensor_scalar_mul(pack[:, SCAL_COL + 1:SCAL_COL + 2],
                                        zred[:, 1:2], inv_nn1)
            nc.vector.tensor_copy(
                pack[:, GCOLS + COFF[5]:GCOLS + COFF[5] + WIDTHS[5]],
                slabs[5][:, 0:WIDTHS[5]])

            # ---- stage cin to DRAM in 4 chunks as packs complete ----
            cin = dram.tile([P, PACK_COLS], F16)
            cout = dram.tile([P, PACK_COLS], F16, addr_space="Shared")
            C1 = COFF[3]                    # x m0..m2  = cols [0, 1920)
            C2 = GCOLS                      # x m3..m5  = [1920, 2688)
            C3 = GCOLS + COFF[3]            # y m0..m2  = [2688, 4608)
            nc.sync.dma_start(cin[:, 0:C1], pack[:, 0:C1])
            nc.sync.dma_start(cin[:, C1:C2], pack[:, C1:C2])
            nc.sync.dma_start(cin[:, C2:C3], pack[:, C2:C3])
            nc.sync.dma_start(cin[:, C3:PACK_COLS], pack[:, C3:PACK_COLS])

            # ---- single fp16 AllReduce of all partials ----
            nc.gpsimd.collective_compute(
                "AllReduce",
                mybir.AluOpType.add,
                replica_groups=[list(range(N_CORES))],
                ins=[cin.opt()],
                outs=[cout.opt()],
            )

            # ---- chunked readback, x/y halves paired for the dot ----
            gsum = sb.tile([P, PACK_COLS], F16)
            nc.sync.dma_start(gsum[:, 0:C1], cout[:, 0:C1])
            nc.sync.dma_start(gsum[:, C2:C3], cout[:, C2:C3])
            nc.sync.dma_start(gsum[:, C1:C2], cout[:, C1:C2])
            nc.sync.dma_start(gsum[:, C3:PACK_COLS], cout[:, C3:PACK_COLS])

            # ---- dot(G_x, G_y) on vector: diag tiles once, strict-upper
            # twice (Grams are symmetric); m0..m2 first (their chunks land
            # first) ----
            dscr = sb.tile([P, 640], F32)
            dcols = sb.tile([P, 2 * MS - 1], F32)  # [0:6] diag, [6:11] upper
            for m in range(MS):
                a, b = COFF[m], GCOLS + COFF[m]
                nc.vector.scalar_tensor_tensor(
                    dscr[:, 0:P], gsum[:, a:a + P], 1.0, gsum[:, b:b + P],
                    mult, mult, accum_out=dcols[:, m:m + 1],
                )
                if m < MS - 1:
                    w = WIDTHS[m] - P
                    nc.vector.scalar_tensor_tensor(
                        dscr[:, 0:w], gsum[:, a + P:a + P + w], 1.0,
                        gsum[:, b + P:b + P + w],
                        mult, mult, accum_out=dcols[:, MS + m:MS + m + 1],
                    )

            # combine: dtot = diag + 2*upper; append the summed z columns
            dred = sb.tile([P, 2], F32)
            nc.vector.tensor_reduce(dred[:, 0:1], dcols[:, 0:MS], AX, add)
            nc.vector.tensor_reduce(dred[:, 1:2], dcols[:, MS:2 * MS - 1], AX, add)
            dtot = sb.tile([P, 3], F32)
            nc.vector.scalar_tensor_tensor(
                dtot[:, 0:1], dred[:, 1:2], 2.0, dred[:, 0:1], mult, add,
            )
            nc.vector.tensor_copy(dtot[:, 1:3], gsum[:, SCAL_COL:SCAL_COL + 2])
            # partition reduction of (dot, s1, s2) in one ones^T matmul
            nc.tensor.matmul(slabs[4][0:1, 0:3], ones[:, 0:1], dtot[:, 0:3],
                             start=True, stop=True)
            dfin = sb.tile([1, 3], F32)
            nc.vector.tensor_copy(dfin[:], slabs[4][0:1, 0:3])
            res = sb.tile([1, 1], F32)
            nc.vector.tensor_scalar_mul(res[:], dfin[:, 0:1], inv_nn1)
            nc.vector.tensor_sub(res[:], res[:], dfin[:, 2:3])
            nc.vector.tensor_sub(res[:], res[:], dfin[:, 1:2])
            nc.sync.dma_start(loss_ap[:], res[:])

    nc.compile()
    return nc


def _get_nc():
    if "nc" not in _CACHE:
        _CACHE["nc"] = _build()
    return _CACHE["nc"]


def _run(x, y, trace=False, **trace_kwargs):
    nc = _get_nc()
    x = np.ascontiguousarray(np.asarray(x, dtype=np.float32))
    y = np.ascontiguousarray(np.asarray(y, dtype=np.float32))
    assert x.shape == (N, D) and y.shape == (N, D)
    in_maps = [
        {"x": x[c * ROWS:(c + 1) * ROWS], "y": y[c * ROWS:(c + 1) * ROWS]}
        for c in range(N_CORES)
    ]
    res = run_bass_kernel_spmd(nc, in_maps, list(range(N_CORES)), trace=trace,
                               **trace_kwargs)
    loss = np.float32(res.results[0]["loss"][0, 0])
    return np.asarray(loss, dtype=np.float32).reshape(()), res


def kernel(x, y):
    out, _ = _run(x, y, trace=False)
    return out


# revision 18
# speedup vs baseline: 1.1204x; 1.1204x over previous
"""L2 (spectral) contrastive loss on 8 Trainium2 NeuronCores.

Math: with G_x = x.T @ x and G_y = y.T @ y (both [D, D]),
    sum_{i,j} <x_i, y_j>^2 = ||x @ y.T||_F^2 = tr(G_x @ G_y) = sum(G_x * G_y)
so the loss needs only the two Gram matrices (2*N*D^2 MACs) instead of the
[N, N] pairwise product (N^2*D MACs) - a 5.3x FLOP reduction at N=8192, D=768.

Sharding: rows of x and y are split across the 8 cores. Each core computes
partial Grams over its 1024 rows (bf16 matmuls, fp32 PSUM accumulation,
upper-triangle slabs only - Grams are symmetric), plus per-partition partial
sums of the diagonal terms z_i = <x_i, y_i>. Partials are packed into one
fp16 buffer [128, 5392] and summed with a single AllReduce; every core then
computes
    loss = sum(G_x*G_y)/(N*(N-1)) - sum(z^2)/(N*(N-1)) - (2/N)*sum(z)
and core 0's output is returned.

Schedule notes (the AllReduce has ~27us of fixed firmware latency plus
~25us of transfer, so everything is arranged to ring its doorbell ASAP):
  - PE has DVFS p-states (full 2.4 GHz only after ~3us of gap-free
    execution), so the matmul queue must never stall once started. Both
    Grams use a diagonal-slot order: slab m consumes chunk k in round k+m,
    so early rounds track the DMA arrival rate, slabs close staggered
    (packs overlap the remaining rounds), and gram-y starts right as
    gram-x drains without waiting for a bulk pack phase.
  - Vector: fp32->bf16 casts of x (PE-critical path), gram-x packs,
    z STTs, the last gram-y pack.
  - Scalar/ACT: casts of y, gram-y packs m0..m4.
  - z partition-reduction is NOT done on-core: the scaled per-partition
    partials ride in two extra pack columns and the AllReduce sums them;
    post-collective they join the dot total in one ones^T matmul.
  - cin is staged to DRAM in 4 column chunks as packs complete, so the
    doorbell waits only on the last slab pack + a ~200KB DMA.
  - Post-collective: 4 chunked readbacks, dot(G_x, G_y) on vector (diag
    tiles once, strict-upper twice), one ones^T matmul reduces the dot
    total and both z scalars across partitions together.
"""
import numpy as np
from contextlib import ExitStack

from concourse import bacc, tile, mybir
from concourse.bass_utils import run_bass_kernel_spmd

N_CORES = 8
N, D = 8192, 768
ROWS = N // N_CORES          # 1024 rows per core
P = 128                      # SBUF partitions
KCH = ROWS // P              # 8 contraction chunks per core
MS = D // P                  # 6 output slabs per Gram

# upper-triangle slab widths and packed column offsets
WIDTHS = [D - P * m for m in range(MS)]              # [768,640,512,384,256,128]
COFF = [sum(WIDTHS[:m]) for m in range(MS)]          # prefix offsets
GCOLS = sum(WIDTHS)                                  # 2688 per Gram
PACK_COLS = 2 * GCOLS + 16                           # + scalar/pad region
SCAL_COL = 2 * GCOLS                                 # z partials at [:, SCAL:+2]
# bank-aligned PSUM slab allocations (bank = 512 fp32)
PSUM_PAD = [1024, 1024, 512, 512, 512, 512]

F32 = mybir.dt.float32
F16 = mybir.dt.float16
BF16 = mybir.dt.bfloat16

_CACHE = {}


def _free_chunks(width):
    """Split [0, width) at the 512-column PSUM bank boundary."""
    if width <= 512:
        return [(0, width)]
    return [(0, 512), (512, width)]


def _build():
    nc = bacc.Bacc("TRN2", target_bir_lowering=False, debug=False,
                   num_devices=N_CORES)
    x_ap = nc.dram_tensor("x", [ROWS, D], F32, kind="ExternalInput").ap()
    y_ap = nc.dram_tensor("y", [ROWS, D], F32, kind="ExternalInput").ap()
    loss_ap = nc.dram_tensor("loss", [1, 1], F32, kind="ExternalOutput").ap()

    inv_nn1 = 1.0 / (float(N) * (N - 1))
    add = mybir.AluOpType.add
    mult = mybir.AluOpType.mult
    AX = mybir.AxisListType.X

    with tile.TileContext(nc) as tc:
        with ExitStack() as ctx:
            sb = ctx.enter_context(tc.tile_pool(name="sb", bufs=1))
            ps = ctx.enter_context(tc.tile_pool(name="ps", bufs=1, space="PSUM"))
            dram = ctx.enter_context(tc.tile_pool(name="dram", bufs=1, space="DRAM"))

            # ---- load inputs: [1024, 768] -> [128p, 8k, 768], x before y ----
            xt = sb.tile([P, KCH, D], F32)
            yt = sb.tile([P, KCH, D], F32)
            xr = x_ap.rearrange("(n p) d -> p n d", p=P)
            yr = y_ap.rearrange("(n p) d -> p n d", p=P)
            for k in range(KCH):
                nc.sync.dma_start(xt[:, k, :], xr[:, k, :])
            for k in range(KCH):
                nc.sync.dma_start(yt[:, k, :], yr[:, k, :])

            # ---- small init on vector ----
            pack = sb.tile([P, PACK_COLS], F16)
            ones = sb.tile([P, 1], F32)
            nc.vector.memset(ones[:], 1.0)
            nc.vector.memset(pack[:, SCAL_COL + 2:PACK_COLS], 0.0)

            # ---- casts to bf16: x on vector (fast, PE-critical),
            # y on the scalar/ACT engine ----
            xb = sb.tile([P, KCH, D], BF16)
            yb = sb.tile([P, KCH, D], BF16)
            for k in range(KCH):
                nc.vector.tensor_copy(xb[:, k, :], xt[:, k, :])
            for k in range(KCH):
                nc.scalar.copy(yb[:, k, :], yt[:, k, :])

            # ---- PSUM slabs: one per m, bank-aligned, all 8 banks ----
            slabs = [
                ps.tile([P, WIDTHS[m]], F32,
                        padded_shape=[P, PSUM_PAD[m]], name=f"slab{m}")
                for m in range(MS)
            ]

            # ---- gram-x: diagonal-slot order (slab m eats chunk k in
            # round k+m) - arrival-paced, staggered closes, no PE stalls ----
            for s in range(KCH + MS - 1):
                for m in range(MS):
                    k = s - m
                    if 0 <= k < KCH:
                        for (c0, c1) in _free_chunks(WIDTHS[m]):
                            nc.tensor.matmul(
                                slabs[m][:, c0:c1],
                                xb[:, k, P * m:P * (m + 1)],
                                xb[:, k, P * m + c0:P * m + c1],
                                start=(k == 0),
                                stop=(k == KCH - 1),
                            )
            # pack gram-x on vector as slabs close
            for m in range(MS):
                nc.vector.tensor_copy(pack[:, COFF[m]:COFF[m] + WIDTHS[m]],
                                      slabs[m][:, 0:WIDTHS[m]])

            # ---- gram-y reusing the same slabs (WAR-ordered after packs) ----
            for s in range(KCH + MS - 1):
                for m in range(MS):
                    k = s - m
                    if 0 <= k < KCH:
                        for (c0, c1) in _free_chunks(WIDTHS[m]):
                            nc.tensor.matmul(
                                slabs[m][:, c0:c1],
                                yb[:, k, P * m:P * (m + 1)],
                                yb[:, k, P * m + c0:P * m + c1],
                                start=(k == 0),
                                stop=(k == KCH - 1),
                            )
            # pack gram-y: m0..m3 on ACT (free after casts), m4/m5 on vector
            # (they gate the last cin chunk; vector copies are faster)
            for m in range(4):
                nc.scalar.copy(pack[:, GCOLS + COFF[m]:GCOLS + COFF[m] + WIDTHS[m]],
                               slabs[m][:, 0:WIDTHS[m]])

            # ---- z partials on vector: zred[p] = (sum_k z, sum_k z^2),
            # scaled and quantized per-partition; the AllReduce sums them ----
            zcols = sb.tile([P, KCH], F32)
            zscr = sb.tile([P, D], F32)
            for k in range(KCH):
                nc.vector.scalar_tensor_tensor(
                    zscr[:], xb[:, k, :], 1.0, yb[:, k, :],
                    mult, mult, accum_out=zcols[:, k:k + 1],
                )
            zsq = sb.tile([P, KCH], F32)
            nc.vector.tensor_mul(zsq[:], zcols[:], zcols[:])
            zred = sb.tile([P, 2], F32)
            nc.vector.tensor_reduce(zred[:, 0:1], zcols[:], AX, add)
            nc.vector.tensor_reduce(zred[:, 1:2], zsq[:], AX, add)
            nc.vector.tensor_scalar_mul(pack[:, SCAL_COL:SCAL_COL + 1],
                                        zred[:, 0:1], 2.0 / N)
            nc.vector.tensor_scalar_mul(pack[:, SCAL_COL + 1:SCAL_COL + 2],
                                        zred[:, 1:2], inv_nn1)
            for m in (4, 5):
                nc.vector.tensor_copy(
                    pack[:, GCOLS + COFF[m]:GCOLS + COFF[m] + WIDTHS[m]],
                    slabs[m][:, 0:WIDTHS[m]])

            # ---- stage cin to DRAM in 4 chunks as packs complete ----
            cin = dram.tile([P, PACK_COLS], F16)
            cout = dram.tile([P, PACK_COLS], F16, addr_space="Shared")
            C1 = COFF[3]                    # x m0..m2  = cols [0, 1920)
            C2 = GCOLS                      # x m3..m5  = [1920, 2688)
            C3 = GCOLS + COFF[3]            # y m0..m2  = [2688, 4608)
            nc.sync.dma_start(cin[:, 0:C1], pack[:, 0:C1])
            nc.sync.dma_start(cin[:, C1:C2], pack[:, C1:C2])
            nc.sync.dma_start(cin[:, C2:C3], pack[:, C2:C3])
            nc.sync.dma_start(cin[:, C3:PACK_COLS], pack[:, C3:PACK_COLS])

            # ---- single fp16 AllReduce of all partials ----
            nc.gpsimd.collective_compute(
                "AllReduce",
                mybir.AluOpType.add,
                replica_groups=[list(range(N_CORES))],
                ins=[cin.opt()],
                outs=[cout.opt()],
            )

            # ---- chunked readback: per-slab (x_m, y_m) pairs in dot order,
            # so the m0 dot can start after ~380KB instead of the full 1.4MB ----
            gsum = sb.tile([P, PACK_COLS], F16)
            for m in range(MS):
                a, b = COFF[m], GCOLS + COFF[m]
                w = WIDTHS[m]
                nc.sync.dma_start(gsum[:, a:a + w], cout[:, a:a + w])
                nc.sync.dma_start(gsum[:, b:b + w], cout[:, b:b + w])
            nc.sync.dma_start(gsum[:, SCAL_COL:SCAL_COL + 2],
                              cout[:, SCAL_COL:SCAL_COL + 2])

            # ---- dot(G_x, G_y) on vector: diag tiles once, strict-upper
            # twice (Grams are symmetric); m0..m2 first (their chunks land
            # first) ----
            dscr = sb.tile([P, 640], F32)
            dcols = sb.tile([P, 2 * MS - 1], F32)  # [0:6] diag, [6:11] upper
            for m in range(MS):
                a, b = COFF[m], GCOLS + COFF[m]
                nc.vector.scalar_tensor_tensor(
                    dscr[:, 0:P], gsum[:, a:a + P], 1.0, gsum[:, b:b + P],
                    mult, mult, accum_out=dcols[:, m:m + 1],
                )
                if m < MS - 1:
                    w = WIDTHS[m] - P
                    nc.vector.scalar_tensor_tensor(
                        dscr[:, 0:w], gsum[:, a + P:a + P + w], 1.0,
                        gsum[:, b + P:b + P + w],
                        mult, mult, accum_out=dcols[:, MS + m:MS + m + 1],
                    )

            # combine: dtot = diag + 2*upper; append the summed z columns
            dred = sb.tile([P, 2], F32)
            nc.vector.tensor_reduce(dred[:, 0:1], dcols[:, 0:MS], AX, add)
            nc.vector.tensor_reduce(dred[:, 1:2], dcols[:, MS:2 * MS - 1], AX, add)
            dtot = sb.tile([P, 3], F32)
            nc.vector.scalar_tensor_tensor(
                dtot[:, 0:1], dred[:, 1:2], 2.0, dred[:, 0:1], mult, add,
            )
            nc.vector.tensor_copy(dtot[:, 1:3], gsum[:, SCAL_COL:SCAL_COL + 2])
            # partition reduction of (dot, s1, s2) in one ones^T matmul
            nc.tensor.matmul(slabs[4][0:1, 0:3], ones[:, 0:1], dtot[:, 0:3],
                             start=True, stop=True)
            dfin = sb.tile([1, 3], F32)
            nc.vector.tensor_copy(dfin[:], slabs[4][0:1, 0:3])
            res = sb.tile([1, 1], F32)
            nc.vector.tensor_scalar_mul(res[:], dfin[:, 0:1], inv_nn1)
            nc.vector.tensor_sub(res[:], res[:], dfin[:, 2:3])
            nc.vector.tensor_sub(res[:], res[:], dfin[:, 1:2])
            nc.sync.dma_start(loss_ap[:], res[:])

    nc.compile()
    return nc


def _get_nc():
    if "nc" not in _CACHE:
        _CACHE["nc"] = _build()
    return _CACHE["nc"]


def _run(x, y, trace=False, **trace_kwargs):
    nc = _get_nc()
    x = np.ascontiguousarray(np.asarray(x, dtype=np.float32))
    y = np.ascontiguousarray(np.asarray(y, dtype=np.float32))
    assert x.shape == (N, D) and y.shape == (N, D)
    in_maps = [
        {"x": x[c * ROWS:(c + 1) * ROWS], "y": y[c * ROWS:(c + 1) * ROWS]}
        for c in range(N_CORES)
    ]
    res = run_bass_kernel_spmd(nc, in_maps, list(range(N_CORES)), trace=trace,
                               **trace_kwargs)
    loss = np.float32(res.results[0]["loss"][0, 0])
    return np.asarray(loss, dtype=np.float32).reshape(()), res


def kernel(x, y):
    out, _ = _run(x, y, trace=False)
    return out
